# revision 27
# baseline (speedup 1.0000x reference)
"""Trainium2 Bass kernel for nn_NodeDetector (masked-node GATv2 ensemble).

v4: one-hot matmul gather/scatter (fp8 one-hot lhsT x bf16 hi/lo value
tables -> fp32-exact), SBUF-resident tables, dst-sorted edge-major
processing (34 tiles of 128 edges), per-dst softmax sums accumulated in
PSUM via fp32 one-hot scatter matmuls.  All inputs arrive in 3 packed
DRAM tensors (128-descriptor DMAs instead of ~5000).

Tricks:
- logit = att.lrelu(u) = 0.6*(a_l[src]+a_r[dst]) + 0.4*att.|u|, the a_*
  per-node scalars ride along as cols 128:130 of the value tables;
  |u| runs on the Scalar engine straight out of PSUM.
- num[d] = sum_e w*u - den[d]*xr[d] (u = xl+xr): only u is gathered;
  num kept NEGATED (den*xr - sum w*u) to fit the fused DVE op.
- phase A computes only the <=128 per-core "rare" light pairs (v,d)
  actually consumed by layer 2, batched as one [128, 2, 130] block for
  the new/old edge stages.

Per core: 32 variants; phases 0/P1 replicated; no collectives.
"""

import numpy as np
import ml_dtypes

import concourse.bass as bass
import concourse.mybir as mybir
import concourse.tile as tile
from concourse import bacc
from concourse.bass_utils import run_bass_kernel_spmd
from concourse.masks import make_identity

F32 = mybir.dt.float32
FP16 = mybir.dt.float16
BF16 = mybir.dt.bfloat16
FP8 = mybir.dt.float8e4
AF = mybir.ActivationFunctionType
OP = mybir.AluOpType
AX = mybir.AxisListType
FP8NP = ml_dtypes.float8_e4m3

N = 256
NH = 2
NCORES = 8
VPC = 32
ET_P1 = 34
W = 130          # f32 table width: 128 cols + 2 attention a-cols
WV = 132         # bf16 gather width: 128 vals + acol hi pair + lo pair
NG = 3           # etiles per DVE group (3*132*4B fits one PSUM bank)


# ------------------------------------------------------------------
# host tables
# ------------------------------------------------------------------

def _build_tables(edge_index):
    src = np.asarray(edge_index[0]).astype(np.int64)
    dst = np.asarray(edge_index[1]).astype(np.int64)
    E = src.shape[0]
    order = np.argsort(dst, kind="stable")
    p1_src, p1_dst = src[order], dst[order]

    p1src8 = np.zeros((128, ET_P1 * 2 * 128), np.float32)
    dst_chunks, sc_halves = [], []
    dst_blocks, sc_blocks = [], []
    for t in range(ET_P1):
        es = slice(128 * t, 128 * (t + 1))
        s_t, d_t = p1_src[es], p1_dst[es]
        for c in range(2):
            m = (s_t // 128) == c
            p1src8[s_t[m] - 128 * c,
                   (2 * t + c) * 128 + np.where(m)[0]] = 1.0
        dl, sl = [], []
        for c in range(2):
            m = (d_t // 128) == c
            if m.any():
                oh = np.zeros((128, 128), np.float32)
                oh[d_t[m] - 128 * c, np.where(m)[0]] = 1.0
                dl.append(c)
                dst_blocks.append(oh)
                sc = np.zeros((128, 128), np.float32)
                sc[np.where(m)[0], d_t[m] - 128 * c] = 1.0
                sl.append(c)
                sc_blocks.append(sc)
        dst_chunks.append(tuple(dl))
        sc_halves.append(tuple(sl))
    p1dst8 = np.concatenate(dst_blocks, axis=1)
    p1sc32 = np.concatenate(sc_blocks, axis=1).astype(np.float32)

    in_edges_of = [np.where((dst == v) & (src != v))[0] for v in range(N)]
    out_cnt = {}
    for e in range(E):
        if src[e] != dst[e]:
            out_cnt.setdefault(int(src[e]), {})
            d = int(dst[e])
            out_cnt[int(src[e])][d] = out_cnt[int(src[e])].get(d, 0) + 1
    m_self = np.array([((src == v) & (dst == v)).sum() for v in range(N)],
                      np.float32)

    pre = []
    EBs = []
    for c in range(NCORES):
        V = list(range(VPC * c, VPC * (c + 1)))
        el = np.concatenate([in_edges_of[v] for v in V])
        el = el[np.argsort(dst[el], kind="stable")]
        in_set = [set(src[in_edges_of[v]].tolist()) for v in V]
        rare = []
        for vi, v in enumerate(V):
            for d in sorted(out_cnt.get(v, {})):
                if d in in_set[vi]:
                    rare.append((vi, d, out_cnt[v][d]))
        assert len(rare) <= 128, f"rare overflow {len(rare)}"
        EBs.append(-(-len(el) // 128))
        pre.append((V, el, rare))
    EB = max(EBs)

    percore = []
    for c in range(NCORES):
        V, el, rare = pre[c]
        nE = len(el)
        b_src = np.zeros((128, EB * 2 * 128), np.float32)
        d_src = np.zeros((128, EB * 3 * 128), np.float32)
        xr_oh = np.zeros((32, EB * 128), np.float32)
        sc_oh = np.zeros((128, EB * 32), np.float32)
        rare_pos = {(vi, d): i for i, (vi, d, _) in enumerate(rare)}
        for t in range(EB):
            for i in range(128):
                k = 128 * t + i
                if k >= nE:
                    continue
                e = el[k]
                s, v = int(src[e]), int(dst[e])
                vi = v - 32 * c
                ch = s // 128
                b_src[s - 128 * ch, (2 * t + ch) * 128 + i] = 1.0
                if (vi, s) in rare_pos:
                    d_src[rare_pos[(vi, s)], (3 * t + 2) * 128 + i] = 1.0
                else:
                    d_src[s - 128 * ch, (3 * t + ch) * 128 + i] = 1.0
                xr_oh[vi, 128 * t + i] = 1.0
                sc_oh[i, 32 * t + vi] = 1.0
        sv = np.zeros((128, 64), np.float32)
        for vi, v in enumerate(V):
            sv[v % 128, 32 * (v // 128) + vi] = 1.0
        a_d = np.zeros((128, 256), np.float32)
        a_xls = np.zeros((32, 128), np.float32)
        a_xl = np.zeros((128, 256), np.float32)
        a_C = np.zeros((128, 1), np.float32)
        for i, (vi, d, cnt) in enumerate(rare):
            a_d[d % 128, 128 * (d // 128) + i] = 1.0
            a_xls[vi, i] = 1.0
            v = V[vi]
            a_xl[v % 128, 128 * (v // 128) + i] = 1.0
            a_C[i, 0] = cnt
        percore.append({
            "bsrc8": b_src, "dsrc8": d_src, "xr8": xr_oh, "bsc32": sc_oh,
            "sv8": sv, "selfdiag": np.diag(m_self[V]).astype(np.float32),
            "a_d8": a_d, "a_xls8": a_xls, "a_xl8": a_xl, "a_C": a_C,
        })

    shared = {"p1src8": p1src8, "p1dst8": p1dst8, "p1sc32": p1sc32}
    dims = dict(EB=EB, dst_chunks=tuple(dst_chunks),
                sc_halves=tuple(sc_halves), n_dst=p1dst8.shape[1] // 128,
                n_sc=p1sc32.shape[1] // 128)
    return shared, percore, dims


def _prep_weights(inp):
    f32 = np.float32
    w = {k: np.asarray(v, f32) for k, v in inp.items() if k != "edge_index"}
    att1, att2 = w["g1_att"], w["g2_att"]

    def acol(wmat, att):
        return np.stack([wmat[:, 64 * h:64 * (h + 1)] @ att[h]
                         for h in range(NH)], axis=1).astype(f32)

    def rep(v):
        v = np.asarray(v, f32).reshape(1, -1)
        return np.ascontiguousarray(np.broadcast_to(v, (128, v.shape[1])))

    blr = w["g2_bl"] + w["g2_br"]
    acb2 = np.stack([blr[64 * h:64 * (h + 1)] @ att2[h] for h in range(NH)])
    acb_l = np.stack([w["g1_bl"][64 * h:64 * (h + 1)] @ att1[h]
                      for h in range(NH)])
    acb_r = np.stack([w["g1_br"][64 * h:64 * (h + 1)] @ att1[h]
                      for h in range(NH)])
    P = {
        "x0": w["x"][0:128], "x1": w["x"][128:256],
        "e0": w["E_emb"][0:128], "e1": w["E_emb"][128:256],
        "w1lra": np.concatenate([acol(w["g1_wl"], att1),
                                 acol(w["g1_wr"], att1)], axis=1),
        "acblr": rep(np.concatenate([acb_l, acb_r])),
        "W2LA": np.concatenate([w["g2_wl"], acol(w["g2_wl"], att2)], axis=1),
        "W2RA": np.concatenate([w["g2_wr"], acol(w["g2_wr"], att2)], axis=1),
        "blra": rep(np.concatenate([blr, acb2])),
        "att1r": rep(np.concatenate([att1[0], att1[1]]) * 0.4),
        "att2r": rep(np.concatenate([att2[0], att2[1]]) * 0.4),
        "g1bias": rep(w["g1_bias"]),
        "g2bias": rep(w["g2_bias"]),
        "conv_b": w["conv_b"].reshape(128, 1),
        "lin2_b": w["lin2_b"].reshape(64, 1),
        "g1_bl": w["g1_bl"].reshape(128, 1),
        "g1_br": w["g1_br"].reshape(128, 1),
        "rec_b": w["rec_b"].reshape(64, 1),
    }
    for nm in ("node_proj", "emb_proj", "conv_w0", "conv_w1", "lin2_w",
               "masked_proj", "normal_proj", "g1_wl", "g1_wr", "rec_w"):
        P[nm] = w[nm]
    return P


# ------------------------------------------------------------------
# input packing (3 DRAM tensors; 128 descriptors per DMA)
# ------------------------------------------------------------------

def _pack_specs(dims):
    EB, n_dst, n_sc = dims["EB"], dims["n_dst"], dims["n_sc"]
    pkw = [("x0", 128, 64), ("x1", 128, 64), ("e0", 128, 64),
           ("e1", 128, 64), ("node_proj", 64, 128), ("emb_proj", 64, 128),
           ("conv_w0", 128, 128), ("conv_w1", 128, 128), ("conv_b", 128, 1),
           ("lin2_w", 128, 64), ("lin2_b", 64, 1), ("masked_proj", 64, 64),
           ("normal_proj", 64, 64), ("g1_wl", 64, 128), ("g1_bl", 128, 1),
           ("g1_wr", 64, 128), ("g1_br", 128, 1), ("w1lra", 64, 4),
           ("acblr", 128, 4), ("W2LA", 64, W), ("W2RA", 64, W),
           ("blra", 128, W), ("att1r", 128, 128), ("att2r", 128, 128),
           ("g1bias", 128, 64), ("g2bias", 128, 64), ("rec_w", 64, 64),
           ("rec_b", 64, 1), ("selfdiag", 32, 32), ("a_C", 128, 1)]
    pk8 = [("sv8", 128, 64), ("xr8", 32, EB * 128),
           ("bsrc8", 128, EB * 2 * 128), ("p1src8", 128, ET_P1 * 2 * 128),
           ("p1dst8", 128, n_dst * 128), ("dsrc8", 128, EB * 3 * 128),
           ("a_d8", 128, 256), ("a_xls8", 32, 128), ("a_xl8", 128, 256)]
    pkf = [("bsc32", 128, EB * 32), ("p1sc32", 128, n_sc * 128)]
    return pkw, pk8, pkf


def _pack_offsets(spec):
    off, pos = {}, 0
    for name, rows, cols in spec:
        off[name] = (rows, pos, cols)
        pos += cols
    return off, pos


def _pack_arrays(spec, src_dict, np_dtype):
    off, total = _pack_offsets(spec)
    arr = np.zeros((128, total), np_dtype)
    for name, rows, cols in spec:
        v = np.asarray(src_dict[name], np.float32)
        assert v.shape == (rows, cols), (name, v.shape, rows, cols)
        arr[0:rows, off[name][1]:off[name][1] + cols] = v.astype(np_dtype)
    return arr


# ------------------------------------------------------------------
# device program
# ------------------------------------------------------------------

def _build_program(dims, dbg=False):
    nc = bacc.Bacc("TRN2", target_bir_lowering=False, debug=False)
    pkw, pk8, pkf = _pack_specs(dims)
    D = {"_dbg": dbg}
    D["PKW"] = nc.dram_tensor("PKW", [128, _pack_offsets(pkw)[1]], F32,
                              kind="ExternalInput")
    D["PK8"] = nc.dram_tensor("PK8", [128, _pack_offsets(pk8)[1]], FP8,
                              kind="ExternalInput")
    D["PKF"] = nc.dram_tensor("PKF", [128, _pack_offsets(pkf)[1]], F32,
                              kind="ExternalInput")
    D["outT"] = nc.dram_tensor("outT", [64, VPC], F32, kind="ExternalOutput")
    with tile.TileContext(nc) as tc:
        _trace(nc, tc, D, dims)
    nc.compile()
    return nc


def _trace(nc, tc, D, dims):
    import contextlib
    EB = dims["EB"]
    dst_chunks = dims["dst_chunks"]
    sc_halves = dims["sc_halves"]
    pkw_spec, pk8_spec, pkf_spec = _pack_specs(dims)
    pkw_off = _pack_offsets(pkw_spec)[0]
    pk8_off = _pack_offsets(pk8_spec)[0]
    pkf_off = _pack_offsets(pkf_spec)[0]

    ctx = contextlib.ExitStack()
    with ctx:
        consts = ctx.enter_context(tc.tile_pool(name="consts", bufs=1))
        tabs = ctx.enter_context(tc.tile_pool(name="tabs", bufs=1))
        work = ctx.enter_context(tc.tile_pool(name="work", bufs=3))
        psacc = ctx.enter_context(tc.tile_pool(name="psacc", bufs=1,
                                               space="PSUM"))
        psum = ctx.enter_context(tc.tile_pool(name="psum", bufs=2,
                                              space="PSUM"))

        dma = nc.sync.dma_start
        tt = nc.vector.tensor_tensor
        stt = nc.vector.scalar_tensor_tensor
        red = nc.vector.tensor_reduce
        act = nc.scalar.activation
        mm = nc.tensor.matmul

        def dbg_dump(name, ap):
            if not D.get("_dbg"):
                return
            t_ = nc.dram_tensor("dbg_" + name, list(ap.shape), F32,
                                kind="ExternalOutput")
            dma(out=t_[:], in_=ap)

        ident = consts.tile([128, 128], F32, tag="ident")
        make_identity(nc, ident[:])

        # ---- packed input loads ----
        PKWt = consts.tile([128, D["PKW"].shape[1]], F32, tag="PKW")
        dma(out=PKWt[:], in_=D["PKW"][:])
        PK8t = consts.tile([128, D["PK8"].shape[1]], FP8, tag="PK8")
        dma(out=PK8t[:], in_=D["PK8"][:])
        PKFt = consts.tile([128, D["PKF"].shape[1]], F32, tag="PKF")
        dma(out=PKFt[:], in_=D["PKF"][:])

        def gw(name):
            r, c0, c = pkw_off[name]
            return PKWt[0:r, c0:c0 + c]

        def g8(name):
            r, c0, c = pk8_off[name]
            return PK8t[0:r, c0:c0 + c]

        def gf(name):
            r, c0, c = pkf_off[name]
            return PKFt[0:r, c0:c0 + c]

        att1r = gw("att1r")
        att2r = gw("att2r")
        g1bias = gw("g1bias")
        g2bias = gw("g2bias")
        w2la = gw("W2LA")
        w2ra = gw("W2RA")

        # ---------------- helpers ----------------
        def ts_mul(out, in0, s):
            nc.vector.tensor_scalar_mul(out=out, in0=in0, scalar1=s)

        def mk132(src130, tag, blocks=1, dtype=FP16, with_lo=False):
            """[P, blocks*130] f32 -> [P, blocks*132] fp16/bf16 with
            exact acol hi/lo pairs (cols 128:130 hi, 130:132 lo).
            with_lo: also return a bf16 residual table (val cols only;
            acol cols zero) so hi+lo gather-accumulate is fp32-exact."""
            P = src130.shape[0]
            out = tabs.tile([P, blocks * WV], dtype, tag=tag + "_b16")
            ov = out[:].rearrange("p (b c) -> p b c", b=blocks)
            sv = src130.rearrange("p (b c) -> p b c", b=blocks)
            lo32 = work.tile([P, blocks, 2], F32, tag=tag + "_lo32")
            nc.vector.tensor_copy(out=ov[:, :, 0:130], in_=sv)
            tt(out=lo32[:P, :, :], in0=sv[:, :, 128:130],
               in1=ov[:, :, 128:130], op=OP.subtract)
            nc.vector.tensor_copy(out=ov[:, :, 130:132], in_=lo32[:P, :, :])
            if not with_lo:
                return out
            lo = tabs.tile([P, blocks * WV], BF16, tag=tag + "_rlo")
            lv = lo[:].rearrange("p (b c) -> p b c", b=blocks)
            lr32 = work.tile([P, blocks, 128], F32, tag=tag + "_lr32")
            tt(out=lr32[:P, :, :], in0=sv[:, :, 0:128],
               in1=ov[:, :, 0:128], op=OP.subtract)
            nc.vector.tensor_copy(out=lv[:, :, 0:128], in_=lr32[:P, :, :])
            nc.vector.memset(lv[:, :, 128:132], 0.0)
            return out, lo

        def elu(x_ap, R, tag):
            xp = work.tile([R, 64], F32, tag=tag + "_xp")
            nc.vector.tensor_scalar_max(out=xp[:], in0=x_ap, scalar1=0.0)
            nc.vector.tensor_scalar_min(out=x_ap, in0=x_ap, scalar1=0.0)
            act(out=x_ap, in_=x_ap, func=AF.Exp)
            nc.vector.tensor_scalar_add(out=x_ap, in0=x_ap, scalar1=-1.0)
            tt(out=x_ap, in0=x_ap, in1=xp[:], op=OP.add)

        def mm_to_sbuf(lhsT, rhs, M, Nf, tag, bias=None, func=AF.Identity,
                       extra=None):
            out_t = tabs.tile([M, Nf], F32, tag=tag)
            ps = psum.tile([128, 256], F32, tag="ps")
            mm(ps[:M, :Nf], lhsT, rhs, start=True, stop=extra is None)
            if extra is not None:
                mm(ps[:M, :Nf], extra[0], extra[1], start=False, stop=True)
            if bias is None:
                act(out=out_t[:], in_=ps[:M, :Nf], func=func)
            else:
                act(out=out_t[:], in_=ps[:M, :Nf], func=func, bias=bias)
            return out_t

        # ---------------- phase 0 ----------------
        xT = tabs.tile([64, 256], F32, tag="xT")
        eT = tabs.tile([64, 256], F32, tag="eT")
        for h in range(2):
            for (nm, dstT) in (("x%d" % h, xT), ("e%d" % h, eT)):
                pst = psum.tile([64, 128], F32, tag="ps")
                nc.tensor.transpose(pst[:], gw(nm), ident[:])
                nc.vector.tensor_copy(out=dstT[:, 128 * h:128 * (h + 1)],
                                      in_=pst[:])

        xpT = mm_to_sbuf(gw("node_proj"), xT[:], 128, 256, "xpT")
        epT = mm_to_sbuf(gw("emb_proj"), eT[:], 128, 256, "epT")
        HbT = mm_to_sbuf(gw("conv_w0"), epT[:], 128, 256, "HbT",
                         bias=gw("conv_b"), func=AF.Tanh,
                         extra=(gw("conv_w1"), xpT[:]))
        HsT = mm_to_sbuf(gw("conv_w0"), epT[:], 128, 256, "HsT",
                         bias=gw("conv_b"), func=AF.Tanh)
        MbT = mm_to_sbuf(gw("lin2_w"), HbT[:], 64, 256, "MbT",
                         bias=gw("lin2_b"))
        MsT = mm_to_sbuf(gw("lin2_w"), HsT[:], 64, 256, "MsT",
                         bias=gw("lin2_b"))
        PbT = mm_to_sbuf(gw("normal_proj"), MbT[:], 64, 256, "PbT")
        PsT = mm_to_sbuf(gw("masked_proj"), MsT[:], 64, 256, "PsT")

        VT = tabs.tile([128, 8 * W], F32, tag="VT")
        OFF = {"XL": 0, "XR": 2 * W, "XLs": 4 * W, "XRs": 6 * W}

        for (kl, kr, PT) in (("XL", "XR", PbT), ("XLs", "XRs", PsT)):
            mainL = mm_to_sbuf(gw("g1_wl"), PT[:], 128, 256, "mainT_" + kl,
                               bias=gw("g1_bl"))
            mainR = mm_to_sbuf(gw("g1_wr"), PT[:], 128, 256, "mainT_" + kr,
                               bias=gw("g1_br"))
            for ch in range(2):
                for key, mainT in ((kl, mainL), (kr, mainR)):
                    ps = psum.tile([128, 128], F32, tag="ps")
                    nc.tensor.transpose(ps[:],
                                        mainT[:, 128 * ch:128 * (ch + 1)],
                                        ident[:])
                    nc.vector.tensor_copy(
                        out=VT[:, OFF[key] + W * ch:OFF[key] + W * ch + 128],
                        in_=ps[:])
                psa = psum.tile([128, 4], F32, tag="ps")
                mm(psa[:], PT[:, 128 * ch:128 * (ch + 1)], gw("w1lra"),
                   start=True, stop=True)
                acsb = work.tile([128, 4], F32, tag="acsb")
                tt(out=acsb[:], in0=psa[:], in1=gw("acblr"), op=OP.add)
                nc.vector.tensor_copy(
                    out=VT[:, OFF[kl] + W * ch + 128:OFF[kl] + W * ch + W],
                    in_=acsb[:, 0:2])
                nc.vector.tensor_copy(
                    out=VT[:, OFF[kr] + W * ch + 128:OFF[kr] + W * ch + W],
                    in_=acsb[:, 2:4])

        dbg_dump("VT", VT[:])
        VTB, VTBlo = mk132(VT[:], "VT", blocks=8, with_lo=True)
        OFFV = {k: (OFF[k] // W) * WV for k in OFF}

        def vslice(key, ch, t_=None):
            t_ = VTB if t_ is None else t_
            return t_[:, OFFV[key] + WV * ch:OFFV[key] + WV * (ch + 1)]

        def vpair(key, ch):
            return (vslice(key, ch), vslice(key, ch, VTBlo))

        # ---------------- edge machinery ----------------
        def gather_seq(sl, pairs, start=True, stop=True):
            seq = []
            for p in pairs:
                lhs = p[0]
                for rhs_ap in p[1:]:
                    if rhs_ap is not None:
                        seq.append((lhs, rhs_ap))
            n = len(seq)
            for j, (lhs, rhs_ap) in enumerate(seq):
                mm(sl, lhs, rhs_ap, start=(j == 0 and start),
                   stop=(j == n - 1 and stop), skip_group_check=True)

        def group_stage(ps_u, ng, tag, att_rep):
            psv = ps_u[:].rearrange("p (i c) -> p i c", i=NG)
            absu = work.tile([128, NG, 128], F32, tag=tag + "_absu")
            act(out=absu[:, :ng, :], in_=psv[:, :ng, 0:128], func=AF.Abs)
            tt(out=absu[:, :ng, :], in0=absu[:, :ng, :],
               in1=att_rep.rearrange("p c -> p () c")
               .to_broadcast([128, ng, 128]), op=OP.mult)
            lgabs = work.tile([128, NG, 2], F32, tag=tag + "_lgabs")
            red(out=lgabs[:, :ng, :],
                in_=absu[:, :ng, :].rearrange("p i (h f) -> p i h f", h=2),
                axis=AX.X, op=OP.add)
            logit = work.tile([128, NG, 2], F32, tag=tag + "_logit")
            stt(out=logit[:, :ng, :], in0=psv[:, :ng, 128:130],
                scalar=0.6, in1=lgabs[:, :ng, :], op0=OP.mult, op1=OP.add)
            stt(out=logit[:, :ng, :], in0=psv[:, :ng, 130:132],
                scalar=0.6, in1=logit[:, :ng, :], op0=OP.mult, op1=OP.add)
            wexp = work.tile([128, NG, 2], F32, tag=tag + "_wexp")
            act(out=wexp[:, :ng, :], in_=logit[:, :ng, :], func=AF.Exp)
            rhs = work.tile([128, NG, W], F32, tag=tag + "_rhs")
            tt(out=rhs[:, :ng, 0:128].rearrange("p i (h f) -> p i h f", h=2),
               in0=psv[:, :ng, 0:128].rearrange("p i (h f) -> p i h f", h=2),
               in1=wexp[:, :ng, :].rearrange("p i h -> p i h ()")
               .to_broadcast([128, ng, 2, 64]), op=OP.mult)
            nc.vector.tensor_copy(out=rhs[:, :ng, 128:130],
                                  in_=wexp[:, :ng, :])
            return rhs

        def run_groups(tag, n_et, gather_emit, scatter_emit, att_rep):
            # software-pipelined: group g+1's gathers are emitted before
            # group g's scatters so the tensor queue never stalls on the
            # DVE edge-stage.  psu ring (bufs=3) keeps buffers distinct.
            prev = None
            for g0 in range(0, n_et, NG):
                ng = min(NG, n_et - g0)
                ps_u = psum.tile([128, NG * WV], F32, tag="psu", bufs=3)
                for i in range(ng):
                    gather_emit(g0 + i, ps_u[:, WV * i:WV * (i + 1)])
                if prev is not None:
                    pg0, png, prhs = prev
                    for i in range(png):
                        scatter_emit(pg0 + i, prhs[:, i, :])
                rhs = group_stage(ps_u, ng, tag, att_rep)
                prev = (g0, ng, rhs)
            pg0, png, prhs = prev
            for i in range(png):
                scatter_emit(pg0 + i, prhs[:, i, :])

        def edge_stage_small(u_sb, R, att_rep, tag):
            absu = work.tile([R, 128], F32, tag=tag + "_absu")
            act(out=absu[:], in_=u_sb[:R, 0:128], func=AF.Abs)
            tt(out=absu[:], in0=absu[:], in1=att_rep[:R, :], op=OP.mult)
            lgabs = work.tile([R, 2], F32, tag=tag + "_lgabs")
            red(out=lgabs[:], in_=absu[:].rearrange("p (h f) -> p h f", h=2),
                axis=AX.X, op=OP.add)
            wexp = work.tile([R, 2], F32, tag=tag + "_wexp")
            stt(out=wexp[:], in0=u_sb[:R, 128:130], scalar=0.6,
                in1=lgabs[:], op0=OP.mult, op1=OP.add)
            act(out=wexp[:], in_=wexp[:], func=AF.Exp)
            rhs = work.tile([R, W], F32, tag=tag + "_rhs")
            for h in range(NH):
                tt(out=rhs[:, 64 * h:64 * (h + 1)],
                   in0=u_sb[:R, 64 * h:64 * (h + 1)],
                   in1=wexp[:, h:h + 1].to_broadcast([R, 64]), op=OP.mult)
            nc.vector.tensor_copy(out=rhs[:, 128:130], in_=wexp[:])
            return rhs

        def nd_post(ps_acc, xr_sb, bias_rep, R, tag):
            den = work.tile([R, 2], F32, tag=tag + "_den")
            act(out=den[:], in_=ps_acc[:R, 128:130], func=AF.Copy)
            nn = work.tile([R, 128], F32, tag=tag + "_nn")
            for hd in range(NH):
                stt(out=nn[:, 64 * hd:64 * (hd + 1)],
                    in0=xr_sb[:R, 64 * hd:64 * (hd + 1)],
                    scalar=den[:, hd:hd + 1],
                    in1=ps_acc[:R, 64 * hd:64 * (hd + 1)],
                    op0=OP.mult, op1=OP.subtract)
            recm = work.tile([R, 2], F32, tag=tag + "_recm")
            nc.vector.reciprocal(out=recm[:], in_=den[:])
            ts_mul(recm[:], recm[:], -0.5)
            g = tabs.tile([R, 64], F32, tag=tag + "_g")
            r1 = work.tile([R, 64], F32, tag=tag + "_r1")
            ts_mul(g[:], nn[:, 0:64], recm[:, 0:1])
            ts_mul(r1[:], nn[:, 64:128], recm[:, 1:2])
            tt(out=g[:], in0=g[:], in1=r1[:], op=OP.add)
            tt(out=g[:], in0=g[:], in1=bias_rep[:R, :], op=OP.add)
            elu(g[:], R, tag + "_elu")
            return g

        # ---------------- minis ----------------
        def extract_mini(key, tag):
            ps = psum.tile([32, WV], F32, tag="ps")
            gather_seq(ps[:], [(g8("sv8")[:, 32 * ch:32 * (ch + 1)],) +
                               vpair(key, ch) for ch in range(2)])
            m_ = tabs.tile([32, W], F32, tag=tag)
            nc.vector.tensor_copy(out=m_[:], in_=ps[:, 0:130])
            tt(out=m_[:, 128:130], in0=ps[:, 130:132], in1=m_[:, 128:130],
               op=OP.add)
            return m_

        XLsm = extract_mini("XLs", "XLsm")
        XLsm_b, XLsm_blo = mk132(XLsm[:], "XLsm", with_lo=True)
        XRsm = extract_mini("XRs", "XRsm")
        XRsm_b, XRsm_blo = mk132(XRsm[:], "XRsm", with_lo=True)
        dbg_dump("XLsm", XLsm[:])
        dbg_dump("XRsm", XRsm[:])

        # ---------------- B ----------------
        ps_bd = psacc.tile([32, 2 * W], F32, tag="ps_bd")
        ps_b = ps_bd[:, 0:W]
        ps_d = ps_bd[:, W:2 * W]
        u_self = tabs.tile([32, W], F32, tag="u_self")
        tt(out=u_self[:], in0=XLsm[:], in1=XRsm[:], op=OP.add)
        rhsS = edge_stage_small(u_self, 32, att1r, "bself")
        mm(ps_b, gw("selfdiag"), rhsS[:], start=True, stop=False,
           skip_group_check=True)

        bsrc8 = g8("bsrc8")
        xr8 = g8("xr8")
        bsc32 = gf("bsc32")

        def b_gather(t, sl):
            pairs = [(bsrc8[:, (2 * t + ch) * 128:(2 * t + ch + 1) * 128],) +
                     vpair("XL", ch) for ch in range(2)]
            pairs.append((xr8[:, 128 * t:128 * (t + 1)], XRsm_b[:],
                          XRsm_blo[:]))
            gather_seq(sl, pairs)

        def b_scatter(t, rhs_ap):
            mm(ps_b, bsc32[:, 32 * t:32 * (t + 1)], rhs_ap,
               start=False, stop=(t == EB - 1), skip_group_check=True)

        run_groups("B", EB, b_gather, b_scatter, att1r)
        g1self = nd_post(ps_b, XRsm[:], g1bias, 32, "bpost")
        dbg_dump("g1self", g1self[:])

        # ---------------- XR2S / T2self / D-self (early) --------------
        ps = psum.tile([64, 32], F32, tag="ps")
        nc.tensor.transpose(ps[:], g1self[:], ident[:32, :32])
        g1sT = work.tile([64, 32], F32, tag="g1sT")
        nc.vector.tensor_copy(out=g1sT[:], in_=ps[:])
        ps2 = psum.tile([32, W], F32, tag="ps")
        mm(ps2[:], g1sT[:], w2ra, start=True, stop=True)
        XR2S = tabs.tile([32, W], F32, tag="XR2S")
        tt(out=XR2S[:], in0=ps2[:], in1=gw("blra")[0:32, :], op=OP.add)
        dbg_dump("XR2S", XR2S[:])
        XR2S_b, XR2S_blo = mk132(XR2S[:], "XR2S", with_lo=True)
        ps3 = psum.tile([32, W], F32, tag="ps")
        mm(ps3[:], g1sT[:], w2la, start=True, stop=True)
        u_ds = tabs.tile([32, W], F32, tag="u_ds")
        tt(out=u_ds[:], in0=ps3[:], in1=XR2S[:], op=OP.add)
        rhsS2 = edge_stage_small(u_ds, 32, att2r, "dself")
        mm(ps_d, gw("selfdiag"), rhsS2[:], start=True, stop=False,
           skip_group_check=True)

        # ---------------- P1 ----------------
        ps_num = psacc.tile([128, 2 * W], F32, tag="ps_num")
        p1src8 = g8("p1src8")
        p1dst8 = g8("p1dst8")
        p1sc32 = gf("p1sc32")
        dst_pos, pos = {}, 0
        for t in range(ET_P1):
            for c in dst_chunks[t]:
                dst_pos[(t, c)] = pos
                pos += 1
        sc_pos, pos = {}, 0
        for t in range(ET_P1):
            for h in sc_halves[t]:
                sc_pos[(t, h)] = pos
                pos += 1
        first_h = {h: min(t for t in range(ET_P1) if h in sc_halves[t])
                   for h in range(2)}
        last_h = {h: max(t for t in range(ET_P1) if h in sc_halves[t])
                  for h in range(2)}

        def p1_gather(t, sl):
            pairs = [(p1src8[:, (2 * t + ch) * 128:(2 * t + ch + 1) * 128],)
                     + vpair("XL", ch) for ch in range(2)]
            for c in dst_chunks[t]:
                j = dst_pos[(t, c)]
                pairs.append((p1dst8[:, 128 * j:128 * (j + 1)],)
                             + vpair("XR", c))
            gather_seq(sl, pairs)

        def p1_scatter(t, rhs_ap):
            for h in sc_halves[t]:
                j = sc_pos[(t, h)]
                mm(ps_num[:, W * h:W * (h + 1)],
                   p1sc32[:, 128 * j:128 * (j + 1)], rhs_ap,
                   start=(t == first_h[h]), stop=(t == last_h[h]),
                   skip_group_check=True)

        run_groups("P1", ET_P1, p1_gather, p1_scatter, att1r)

        # ---------------- T1N assembly + hi/lo ------------------------
        T1N = tabs.tile([128, 2 * W], F32, tag="T1N")
        for h in range(2):
            pna = ps_num[:, W * h:W * (h + 1)]
            act(out=T1N[:, W * h + 128:W * h + 130], in_=pna[:, 128:130],
                func=AF.Copy)
            xroff = OFF["XR"] + W * h
            for hd in range(NH):
                stt(out=T1N[:, W * h + 64 * hd:W * h + 64 * (hd + 1)],
                    in0=VT[:, xroff + 64 * hd:xroff + 64 * (hd + 1)],
                    scalar=T1N[:, W * h + 128 + hd:W * h + 129 + hd],
                    in1=pna[:, 64 * hd:64 * (hd + 1)],
                    op0=OP.mult, op1=OP.subtract)
        dbg_dump("T1N", T1N[:])
        T1NB, T1NBlo = mk132(T1N[:], "T1N", blocks=2, dtype=BF16, with_lo=True)

        # ---------------- A gathers + u_no ---------------------------
        a_d8 = g8("a_d8")
        ps_a1 = psum.tile([128, NG * WV], F32, tag="psu", bufs=3)
        gather_seq(ps_a1[:, 0:WV],
                   [(a_d8[:, 128 * ch:128 * (ch + 1)],) + vpair("XR", ch)
                    for ch in range(2)])
        gather_seq(ps_a1[:, WV:2 * WV],
                   [(a_d8[:, 128 * ch:128 * (ch + 1)],
                     T1NB[:, WV * ch:WV * (ch + 1)]) for ch in range(2)] +
                   [(a_d8[:, 128 * ch:128 * (ch + 1)],
                     T1NBlo[:, WV * ch:WV * (ch + 1)]) for ch in range(2)])
        ps_a2 = psum.tile([128, NG * WV], F32, tag="psu", bufs=3)
        gather_seq(ps_a2[:, 0:WV], [(g8("a_xls8"), XLsm_b[:],
                                     XLsm_blo[:])])
        gather_seq(ps_a2[:, WV:2 * WV],
                   [(g8("a_xl8")[:, 128 * ch:128 * (ch + 1)],) +
                    vpair("XL", ch) for ch in range(2)])
        xr_d = work.tile([128, WV], F32, tag="a_xrd")
        act(out=xr_d[:], in_=ps_a1[:, 0:WV], func=AF.Copy)
        # u_no [128, 2, WV]: row 0 = u_new (xls+xr), row 1 = u_old (xl+xr)
        u_no = work.tile([128, 2, WV], F32, tag="a_uno")
        tt(out=u_no[:],
           in0=ps_a2[:].rearrange("p (i c) -> p i c", i=NG)[:, 0:2, :],
           in1=xr_d[:].rearrange("p c -> p () c")
           .to_broadcast([128, 2, WV]), op=OP.add)

        # ---------------- g1 base + T2base + hi/lo --------------------
        T2B = tabs.tile([128, 2 * W], F32, tag="T2B")
        t1v = T1N[:].rearrange("p (b c) -> p b c", b=2)
        recm2 = work.tile([128, 2, 2], F32, tag="g1b_recm")
        nc.vector.reciprocal(out=recm2[:], in_=t1v[:, :, 128:130])
        ts_mul(recm2[:], recm2[:], -0.5)
        g1b2 = work.tile([128, 2, 64], F32, tag="g1b_g")
        r12 = work.tile([128, 2, 64], F32, tag="g1b_r1")
        tt(out=g1b2[:], in0=t1v[:, :, 0:64],
           in1=recm2[:, :, 0:1].to_broadcast([128, 2, 64]), op=OP.mult)
        tt(out=r12[:], in0=t1v[:, :, 64:128],
           in1=recm2[:, :, 1:2].to_broadcast([128, 2, 64]), op=OP.mult)
        tt(out=g1b2[:], in0=g1b2[:], in1=r12[:], op=OP.add)
        tt(out=g1b2[:], in0=g1b2[:],
           in1=g1bias.rearrange("p c -> p () c").to_broadcast([128, 2, 64]),
           op=OP.add)
        xp2 = work.tile([128, 2, 64], F32, tag="g1b_xp")
        nc.vector.tensor_scalar_max(out=xp2[:], in0=g1b2[:], scalar1=0.0)
        nc.vector.tensor_scalar_min(out=g1b2[:], in0=g1b2[:], scalar1=0.0)
        act(out=g1b2[:], in_=g1b2[:], func=AF.Exp)
        nc.vector.tensor_scalar_add(out=g1b2[:], in0=g1b2[:], scalar1=-1.0)
        tt(out=g1b2[:], in0=g1b2[:], in1=xp2[:], op=OP.add)
        for h in range(2):
            ps = psum.tile([64, 128], F32, tag="ps")
            nc.tensor.transpose(ps[:], g1b2[:, h, :], ident[:])
            gT = work.tile([64, 128], F32, tag="g1bT")
            nc.vector.tensor_copy(out=gT[:], in_=ps[:])
            ps2 = psum.tile([128, W], F32, tag="ps")
            mm(ps2[:], gT[:], w2la, start=True, stop=True)
            nc.vector.tensor_copy(out=T2B[:, W * h:W * (h + 1)], in_=ps2[:])
        dbg_dump("T2B", T2B[:])
        T2BB, T2BBlo = mk132(T2B[:], "T2B", blocks=2, with_lo=True)

        # ---------------- A: batched edge stages ----------------------
        absu_a = work.tile([128, 2, 128], F32, tag="a_absu")
        act(out=absu_a[:], in_=u_no[:, :, 0:128], func=AF.Abs)
        tt(out=absu_a[:], in0=absu_a[:],
           in1=att1r.rearrange("p c -> p () c").to_broadcast([128, 2, 128]),
           op=OP.mult)
        lgabs_a = work.tile([128, 2, 2], F32, tag="a_lgabs")
        red(out=lgabs_a[:],
            in_=absu_a[:].rearrange("p i (h f) -> p i h f", h=2),
            axis=AX.X, op=OP.add)
        wno = work.tile([128, 2, 2], F32, tag="a_wno")
        stt(out=wno[:], in0=u_no[:, :, 128:130], scalar=0.6, in1=lgabs_a[:],
            op0=OP.mult, op1=OP.add)
        stt(out=wno[:], in0=u_no[:, :, 130:132], scalar=0.6, in1=wno[:],
            op0=OP.mult, op1=OP.add)
        act(out=wno[:], in_=wno[:], func=AF.Exp)
        ts_mul(wno[:], wno[:], gw("a_C"))
        # d = wo*xl - wn*xls ; nn = d + t1n_negnum ; den = t1n_den + wn - wo
        d_a = work.tile([128, 2, 64], F32, tag="a_d")
        nn_a = work.tile([128, 2, 64], F32, tag="a_nn")
        tt(out=d_a[:],
           in0=ps_a2[:, WV:WV + 128].rearrange("p (h f) -> p h f", h=2),
           in1=wno[:, 1, :].rearrange("p h -> p h ()")
           .to_broadcast([128, 2, 64]), op=OP.mult)
        tt(out=nn_a[:],
           in0=ps_a2[:, 0:128].rearrange("p (h f) -> p h f", h=2),
           in1=wno[:, 0, :].rearrange("p h -> p h ()")
           .to_broadcast([128, 2, 64]), op=OP.mult)
        tt(out=d_a[:], in0=d_a[:], in1=nn_a[:], op=OP.subtract)
        tt(out=nn_a[:], in0=d_a[:],
           in1=ps_a1[:, WV:WV + 128].rearrange("p (h f) -> p h f", h=2),
           op=OP.add)
        den_a = work.tile([128, 2], F32, tag="a_den")
        tt(out=den_a[:], in0=wno[:, 0, :], in1=wno[:, 1, :], op=OP.subtract)
        tt(out=den_a[:], in0=den_a[:], in1=ps_a1[:, WV + 128:WV + 130],
           op=OP.add)
        tt(out=den_a[:], in0=den_a[:], in1=ps_a1[:, WV + 130:WV + 132],
           op=OP.add)
        nc.vector.tensor_scalar_max(out=den_a[:], in0=den_a[:],
                                    scalar1=1e-30)
        recm_a = work.tile([128, 2], F32, tag="a_recm")
        nc.vector.reciprocal(out=recm_a[:], in_=den_a[:])
        ts_mul(recm_a[:], recm_a[:], -0.5)
        g1light = tabs.tile([128, 64], F32, tag="g1light")
        r1a = work.tile([128, 64], F32, tag="a_r1")
        ts_mul(g1light[:], nn_a[:, 0, :], recm_a[:, 0:1])
        ts_mul(r1a[:], nn_a[:, 1, :], recm_a[:, 1:2])
        tt(out=g1light[:], in0=g1light[:], in1=r1a[:], op=OP.add)
        tt(out=g1light[:], in0=g1light[:], in1=g1bias[:], op=OP.add)
        elu(g1light[:], 128, "a_elu")
        dbg_dump("g1light", g1light[:])

        ps = psum.tile([64, 128], F32, tag="ps")
        nc.tensor.transpose(ps[:], g1light[:], ident[:])
        gT = work.tile([64, 128], F32, tag="g1lT")
        nc.vector.tensor_copy(out=gT[:], in_=ps[:])
        ps2 = psum.tile([128, W], F32, tag="ps")
        mm(ps2[:], gT[:], w2la, start=True, stop=True)
        T2r = tabs.tile([128, W], F32, tag="T2r")
        nc.vector.tensor_copy(out=T2r[:], in_=ps2[:])
        dbg_dump("T2r", T2r[:])
        T2rB, T2rBlo = mk132(T2r[:], "T2r", with_lo=True)

        # ---------------- D ----------------
        dsrc8 = g8("dsrc8")

        def d_gather(t, sl):
            pairs = [(dsrc8[:, (3 * t + ch) * 128:(3 * t + ch + 1) * 128],
                      T2BB[:, WV * ch:WV * (ch + 1)],
                      T2BBlo[:, WV * ch:WV * (ch + 1)]) for ch in range(2)]
            pairs.append((dsrc8[:, (3 * t + 2) * 128:(3 * t + 3) * 128],
                          T2rB[:], T2rBlo[:]))
            pairs.append((xr8[:, 128 * t:128 * (t + 1)], XR2S_b[:],
                          XR2S_blo[:]))
            gather_seq(sl, pairs)

        def d_scatter(t, rhs_ap):
            mm(ps_d, bsc32[:, 32 * t:32 * (t + 1)], rhs_ap,
               start=False, stop=(t == EB - 1), skip_group_check=True)

        run_groups("DD", EB, d_gather, d_scatter, att2r)
        g2 = nd_post(ps_d, XR2S[:], g2bias, 32, "dpost")
        dbg_dump("g2", g2[:])

        # ---------------- out (transposed; host untransposes) ---------
        ps = psum.tile([64, 32], F32, tag="ps")
        nc.tensor.transpose(ps[:], g2[:], ident[:32, :32])
        g2T = work.tile([64, 32], F32, tag="g2T")
        nc.vector.tensor_copy(out=g2T[:], in_=ps[:])
        pso = psum.tile([64, 32], F32, tag="ps")
        mm(pso[:], gw("rec_w"), g2T[:], start=True, stop=True)
        outT = work.tile([64, 32], F32, tag="outT")
        act(out=outT[:], in_=pso[:], func=AF.Tanh, bias=gw("rec_b"))
        dma(out=D["outT"][:], in_=outT[:])


# ------------------------------------------------------------------
# entry point
# ------------------------------------------------------------------

_CACHE = {}
TRACE = False
LAST_RESULT = None


def kernel(**inputs):
    global LAST_RESULT
    inputs = {k: np.asarray(v) for k, v in inputs.items()}
    shared, percore, dims = _build_tables(inputs["edge_index"])
    P = _prep_weights(inputs)
    key = (dims["EB"], dims["dst_chunks"], dims["sc_halves"])
    if key not in _CACHE:
        _CACHE[key] = _build_program(dims)
    nc = _CACHE[key]
    pkw, pk8, pkf = _pack_specs(dims)
    in_maps = []
    for c in range(NCORES):
        src = dict(P)
        src.update(shared)
        src.update(percore[c])
        in_maps.append({
            "PKW": _pack_arrays(pkw, src, np.float32),
            "PK8": _pack_arrays(pk8, src, FP8NP),
            "PKF": _pack_arrays(pkf, src, np.float32),
        })
    kw = {}
    if TRACE:
        kw = dict(trace=True, trace_cores=list(range(NCORES)))
    res = run_bass_kernel_spmd(nc, in_maps, core_ids=list(range(NCORES)),
                               **kw)
    LAST_RESULT = res
    out = np.concatenate([res.results[c]["outT"].T for c in range(NCORES)],
                         axis=0)
    return out.astype(np.float32)
